# revision 1
# baseline (speedup 1.0000x reference)
"""DCNv2 + BatchNorm + ReLU on 8 TRN2 NeuronCores (self-contained)."""

import sys
sys.path.insert(0, "/opt/trn_rl_repo")
from contextlib import ExitStack
import dataclasses
import numpy as np
import concourse.bass as bass
import concourse.bacc as bacc
import concourse.mybir as mybir
import concourse.tile as tile
from concourse.masks import make_identity

F16 = mybir.dt.float16
F32 = mybir.dt.float32
I16 = mybir.dt.int16
I32 = mybir.dt.int32
AL = mybir.AluOpType
AF = mybir.ActivationFunctionType

C = 128
KK = 9
OH, W = 48, 96
PAD = 4
RH, WP = 56, 104
NPIX = OH * W            # 4608
V = RH * WP              # 5824
NT_TOTAL = 4 * 96 * 96   # BN denominator
SHIFT = 512.0
MAGIC = 8388608.0        # 2^23
NG = 9                   # gather/z groups (512 px each)
GPX = NPIX // NG         # 512
NCHUNK = 36              # 128-px chunks
CPG = 4                  # chunks per group
NS = KK * CPG            # 36 slots per gather
NIDX = NS * C            # 4608 idxs per gather
WRAPC = NIDX // 16       # 288 wrapped columns per group


def build(n_cores=8, num_groups=NG, do_collective=True, debug=False):
    nc = bacc.Bacc("TRN2", target_bir_lowering=False, debug=False,
                   num_devices=n_cores, num_swdge_queues=4)

    xcl_d = nc.dram_tensor("xcl", [V, C], F16, kind="ExternalInput")
    xcp_d = nc.dram_tensor("xcp", [C, V], F16, kind="ExternalInput")
    waux_d = nc.dram_tensor("waux", [C, KK * 27], F16, kind="ExternalInput")
    wbase_d = nc.dram_tensor("wbase", [3, 27], F16, kind="ExternalInput")
    w2_d = nc.dram_tensor("w2", [C, KK * C], F16, kind="ExternalInput")
    grid_d = nc.dram_tensor("grid", [3, 48 * WP], F16, kind="ExternalInput")
    scal_d = nc.dram_tensor("scal", [C, 2], F32, kind="ExternalInput")
    out_d = nc.dram_tensor("out", [C, NPIX], F32, kind="ExternalOutput")
    # DRAM scratch for idx wrap bounce (per row-corner a)
    d_idx = [nc.dram_tensor(f"d_idx{a}", [NG * NIDX], I16, kind="Internal")
             for a in (0, 1)]

    if do_collective:
        st_loc = nc.dram_tensor("st_loc", [C, 2], F32, kind="Internal")
        st_glob = nc.dram_tensor("st_glob", [C, 2], F32, kind="Internal",
                                 addr_space="Shared")
    if debug:
        dbg_aux_d = nc.dram_tensor("dbg_aux", [27, NPIX], F32, kind="ExternalOutput")
        dbg_s00_d = nc.dram_tensor("dbg_s00", [C, NCHUNK * 9], F32, kind="ExternalOutput")
        dbg_w0_d = nc.dram_tensor("dbg_w0", [C, NG * WRAPC], I16, kind="ExternalOutput")
        dbg_xg_d = nc.dram_tensor("dbg_xg", [C, NS * 2 * C], F16, kind="ExternalOutput")
        dbg_z_d = nc.dram_tensor("dbg_z", [C, NPIX], F32, kind="ExternalOutput")

    # overlapping-rows view of xcl for 2-pixel gather elements:
    # row i = elements [128*i, 128*i + 256)
    xcl_ov = dataclasses.replace(xcl_d.ap(), ap=[[C, V - 2], [1, 2 * C]])

    with tile.TileContext(nc) as tc, ExitStack() as ctx:
        cons = ctx.enter_context(tc.tile_pool(name="cons", bufs=1))
        sb = ctx.enter_context(tc.tile_pool(name="sb", bufs=1))
        gat = ctx.enter_context(tc.tile_pool(name="gat", bufs=2))
        mpool = ctx.enter_context(tc.tile_pool(name="mpool", bufs=3))
        ps_aux = ctx.enter_context(tc.tile_pool(name="ps_aux", bufs=2, space="PSUM"))
        ps_tr = ctx.enter_context(tc.tile_pool(name="ps_tr", bufs=2, space="PSUM"))
        ps_ms = ctx.enter_context(tc.tile_pool(name="ps_ms", bufs=2, space="PSUM"))
        ps_z = ctx.enter_context(tc.tile_pool(name="ps_z", bufs=2, space="PSUM"))

        # ---- constants to SBUF ----
        xcp_s = cons.tile([C, V], F16)
        nc.sync.dma_start(xcp_s[:], xcp_d.ap())
        waux_s = cons.tile([C, KK * 27], F16)
        nc.sync.dma_start(waux_s[:], waux_d.ap())
        wbase_s = cons.tile([3, 27], F16)
        nc.sync.dma_start(wbase_s[:], wbase_d.ap())
        w2_s = cons.tile([C, KK * C], F16)
        nc.sync.dma_start(w2_s[:], w2_d.ap())
        grid_s = cons.tile([3, 48 * WP], F16)
        nc.sync.dma_start(grid_s[:], grid_d.ap())
        scal_s = cons.tile([C, 2], F32)
        nc.sync.dma_start(scal_s[:], scal_d.ap())
        ident32 = cons.tile([C, C], F32)
        make_identity(nc, ident32[:])
        ident16 = cons.tile([C, C], F16)
        nc.vector.tensor_copy(ident16[:], ident32[:])

        # ---- stage 1: aux conv ----
        # Conv runs over flat padded rows (104 wide, 4 rows per chunk =
        # N=416); pad-column outputs are garbage and dropped by the strided
        # PSUM->SBUF copy. Matmul rhs stays single-free-dim (walrus rule).
        aux_cp = sb.tile([27, NPIX], F32)
        for cix in range(12):
            a_ps = ps_aux.tile([27, 416], F32, tag="aux")
            for t in range(KK):
                ty, tx = t // 3, t % 3
                off = (PAD - 1 + ty + 4 * cix) * WP + (tx - 1)
                nc.tensor.matmul(a_ps[:], waux_s[:, t * 27:(t + 1) * 27],
                                 xcp_s[:, off:off + 416],
                                 start=(t == 0), stop=False)
            nc.tensor.matmul(a_ps[:], wbase_s[:],
                             grid_s[:, 4 * cix * WP:4 * cix * WP + 416],
                             start=False, stop=True)
            pv = a_ps[:].rearrange("q (r w) -> q r w", w=WP)
            nc.scalar.activation(aux_cp[:, 384 * cix:384 * (cix + 1)],
                                 pv[:, :, PAD:PAD + W], AF.Copy)

        # ---- stage 2: transpose aux to pixel-partition ----
        auxT = sb.tile([C, NCHUNK, 27], F32)
        for j in range(NCHUNK):
            t_ps = ps_tr.tile([C, 27], F32, tag="tr")
            nc.tensor.transpose(t_ps[:], aux_cp[:, j * C:(j + 1) * C],
                                ident32[0:27, 0:27])
            nc.scalar.activation(auxT[:, j:j + 1, :], t_ps[:], AF.Copy)

        # ---- stage 3: aux math ----
        ML = auxT[:, :, 18:27]
        mask = sb.tile([C, NCHUNK, 9], F32)
        nc.scalar.activation(mask[:], ML, AF.Sigmoid)

        syS = sb.tile([C, NCHUNK, 18], F32)   # sy,sx + SHIFT
        nc.vector.tensor_scalar(syS[:], auxT[:, :, 0:18], SHIFT, None, AL.add)
        # y0f = rint(syS - 0.5) via the fp32 magic-number trick (exact,
        # backend-independent):  (x + (2^23 - 0.5)) - 2^23
        y0f = sb.tile([C, NCHUNK, 18], F32)
        nc.vector.tensor_scalar(y0f[:], syS[:], MAGIC - 0.5, MAGIC,
                                AL.add, AL.subtract)
        fy = sb.tile([C, NCHUNK, 18], F32)
        nc.vector.tensor_tensor(fy[:], syS[:], y0f[:], AL.subtract)

        m1 = sb.tile([C, NCHUNK, 9], F32)
        nc.vector.tensor_tensor(m1[:], mask[:], fy[:, :, 0:9], AL.mult)
        m0 = sb.tile([C, NCHUNK, 9], F32)
        nc.vector.tensor_tensor(m0[:], mask[:], m1[:], AL.subtract)
        s01 = sb.tile([C, NCHUNK, 9], F32)
        s00 = sb.tile([C, NCHUNK, 9], F32)
        s11 = sb.tile([C, NCHUNK, 9], F32)
        s10 = sb.tile([C, NCHUNK, 9], F32)
        fx = fy[:, :, 9:18]
        nc.vector.tensor_tensor(s01[:], m0[:], fx, AL.mult)
        nc.vector.tensor_tensor(s11[:], m1[:], fx, AL.mult)
        nc.vector.tensor_tensor(s00[:], m0[:], s01[:], AL.subtract)
        nc.vector.tensor_tensor(s10[:], m1[:], s11[:], AL.subtract)

        # ---- gather indices (f32, exact ints) ----
        basei = sb.tile([C, NCHUNK, 9], F32)
        nc.vector.scalar_tensor_tensor(basei[:], y0f[:, :, 0:9], float(WP),
                                       y0f[:, :, 9:18], AL.mult, AL.add)
        idxf = [sb.tile([C, NG, KK, CPG], F32, name=f"idxf{a}", tag=f"idxf{a}")
                for a in (0, 1)]
        for a in (0, 1):
            ca = float((PAD - SHIFT) * WP + PAD - SHIFT + a * WP)
            t = idxf[a][:]
            dst = dataclasses.replace(t, ap=[t.ap[0], [36, 9], [1, 4], [4, 9]])
            nc.vector.tensor_scalar(dst, basei[:], ca, None, AL.add)
            nc.vector.tensor_scalar(idxf[a][:], idxf[a][:], 0.0, float(V - 3),
                                    AL.max, AL.min)

        # ---- stage 4: idx wrap via PE transpose + DRAM bounce ----
        for a in (0, 1):
            for g in range(num_groups):
                tp = ps_tr.tile([NS, C], F32, tag="tr")
                nc.tensor.transpose(tp[:], idxf[a][:, g:g + 1, :, :],
                                    ident32[:])
                ti = mpool.tile([NS, C], I16, tag="ti")
                nc.vector.tensor_copy(ti[:], tp[:])
                nc.sync.dma_start(
                    dataclasses.replace(d_idx[a].ap(), offset=g * NIDX,
                                        ap=[[C, NS], [1, C]]),
                    ti[:])
        wrap = []
        for a in (0, 1):
            w_a = sb.tile([C, num_groups * WRAPC], I16, name=f"wrap{a}", tag=f"wrap{a}")
            for b in range(8):
                nc.sync.dma_start(
                    w_a[16 * b:16 * (b + 1), :],
                    dataclasses.replace(d_idx[a].ap(),
                                        ap=[[1, 16], [16, num_groups * WRAPC]]))
            wrap.append(w_a)

        # ---- stages 5-7 per group ----
        z_sb = sb.tile([C, NPIX], F32)
        st1 = sb.tile([C, NG], F32)
        st2 = sb.tile([C, NG], F32)
        for g in range(num_groups):
            xg0 = gat.tile([C, NS, 2 * C], F16, tag="xg0")
            xg1 = gat.tile([C, NS, 2 * C], F16, tag="xg1")
            # SWDGE rings top out ~1024 descriptors: split each group's
            # gather into slot-blocks of 8 (1024 idxs), round-robin queues.
            for gi, (xg, w_a) in enumerate(((xg0, wrap[0]), (xg1, wrap[1]))):
                for bi, s0 in enumerate(range(0, NS, 8)):
                    s1 = min(s0 + 8, NS)
                    nn = 128 * (s1 - s0)
                    nc.gpsimd.dma_gather(
                        out_ap=xg[:, s0:s1, :], in_ap=xcl_ov,
                        idxs_ap=w_a[:, g * WRAPC + 8 * s0:g * WRAPC + 8 * s1],
                        num_idxs=nn, num_idxs_reg=nn,
                        elem_size=2 * C, elem_step=C,
                        queue_num=(g * 10 + gi * 5 + bi) % 4)
            if debug and g == 0:
                nc.sync.dma_start(dbg_xg_d.ap(),
                                  xg0[:].rearrange("p a b -> p (a b)"))
            z_ps = ps_z.tile([C, GPX], F32, tag="z")
            for jj in range(CPG):
                jc = g * CPG + jj
                msamp = mpool.tile([C, KK, C], F16, tag="msamp")
                for k in range(KK):
                    s = k * CPG + jj
                    x00 = xg0[:, s:s + 1, 0:C]
                    x01 = xg0[:, s:s + 1, C:2 * C]
                    x10 = xg1[:, s:s + 1, 0:C]
                    x11 = xg1[:, s:s + 1, C:2 * C]
                    mk = msamp[:, k:k + 1, :]
                    nc.vector.tensor_scalar(
                        mk, x00, s00[:, jc:jc + 1, k:k + 1], None, AL.mult)
                    nc.vector.scalar_tensor_tensor(
                        mk, x01, s01[:, jc:jc + 1, k:k + 1], mk, AL.mult, AL.add)
                    nc.vector.scalar_tensor_tensor(
                        mk, x10, s10[:, jc:jc + 1, k:k + 1], mk, AL.mult, AL.add)
                    nc.vector.scalar_tensor_tensor(
                        mk, x11, s11[:, jc:jc + 1, k:k + 1], mk, AL.mult, AL.add)
                msampT = mpool.tile([C, KK, C], F16, tag="msampT")
                for k in range(KK):
                    mt_ps = ps_ms.tile([C, C], F16, tag="ms")
                    nc.tensor.transpose(mt_ps[:], msamp[:, k:k + 1, :], ident16[:])
                    nc.scalar.activation(msampT[:, k:k + 1, :], mt_ps[:], AF.Copy)
                for k in range(KK):
                    nc.tensor.matmul(z_ps[:, jj * C:(jj + 1) * C],
                                     w2_s[:, k * C:(k + 1) * C],
                                     msampT[:, k:k + 1, :],
                                     start=(k == 0), stop=(k == KK - 1))
            # PSUM->SBUF copy fused with per-channel sum; square fused w/ sumsq
            nc.scalar.activation(z_sb[:, g * GPX:(g + 1) * GPX], z_ps[:],
                                 AF.Copy, accum_out=st1[:, g:g + 1])
            sq = mpool.tile([C, GPX], F32, tag="sq")
            nc.scalar.activation(sq[:], z_sb[:, g * GPX:(g + 1) * GPX],
                                 AF.Square, accum_out=st2[:, g:g + 1])

        # ---- stage 8: BN ----
        stat = sb.tile([C, 2], F32)
        nc.vector.tensor_reduce(stat[:, 0:1], st1[:, 0:num_groups],
                                mybir.AxisListType.X, AL.add)
        nc.vector.tensor_reduce(stat[:, 1:2], st2[:, 0:num_groups],
                                mybir.AxisListType.X, AL.add)
        if do_collective:
            nc.sync.dma_start(st_loc.ap(), stat[:])
            nc.gpsimd.collective_compute(
                kind="AllReduce",
                op=AL.add,
                replica_groups=[list(range(n_cores))],
                ins=[st_loc.ap()],
                outs=[st_glob.ap()],
            )
            gstat = sb.tile([C, 2], F32)
            nc.sync.dma_start(gstat[:], st_glob.ap())
        else:
            gstat = stat

        mean = sb.tile([C, 1], F32)
        nc.vector.tensor_scalar(mean[:], gstat[:, 0:1], 1.0 / NT_TOTAL, None,
                                AL.mult)
        var = sb.tile([C, 1], F32)
        nc.vector.tensor_scalar(var[:], gstat[:, 1:2], 1.0 / NT_TOTAL, None,
                                AL.mult)
        msq = sb.tile([C, 1], F32)
        nc.vector.tensor_tensor(msq[:], mean[:], mean[:], AL.mult)
        nc.vector.tensor_tensor(var[:], var[:], msq[:], AL.subtract)
        eps = sb.tile([C, 1], F32)
        nc.gpsimd.memset(eps[:], 1e-5)
        std = sb.tile([C, 1], F32)
        nc.scalar.activation(std[:], var[:], AF.Sqrt, bias=eps[:])
        inv = sb.tile([C, 1], F32)
        nc.vector.reciprocal(inv[:], std[:])
        A = sb.tile([C, 1], F32)
        nc.vector.tensor_tensor(A[:], inv[:], scal_s[:, 0:1], AL.mult)
        B = sb.tile([C, 1], F32)
        nc.vector.tensor_tensor(B[:], mean[:], A[:], AL.mult)
        nc.vector.tensor_tensor(B[:], scal_s[:, 1:2], B[:], AL.subtract)

        out_s = sb.tile([C, NPIX], F32)
        for g in range(num_groups):
            sl = slice(g * GPX, (g + 1) * GPX)
            nc.vector.tensor_scalar(out_s[:, sl], z_sb[:, sl], A[:], B[:],
                                    AL.mult, AL.add)
            nc.vector.tensor_scalar(out_s[:, sl], out_s[:, sl], 0.0, None,
                                    AL.max)
        nc.sync.dma_start(out_d.ap(), out_s[:])
        if debug:
            nc.sync.dma_start(dbg_aux_d.ap(), aux_cp[:])
            nc.sync.dma_start(dbg_s00_d.ap(), s00[:].rearrange("p a b -> p (a b)"))
            nc.sync.dma_start(dbg_w0_d.ap(), wrap[0][:])
            nc.sync.dma_start(dbg_z_d.ap(), z_sb[:])

    nc.compile()
    return nc


# ---------------- PJRT runner ----------------
import sys
sys.path.insert(0, "/opt/trn_rl_repo")
import time
import numpy as np
import jax
import jax.numpy as jnp
from jax.sharding import Mesh, PartitionSpec
from jax.experimental.shard_map import shard_map
import concourse.bass as bass
import concourse.mybir as mybir
from concourse import bass2jax
from concourse.bass2jax import _bass_exec_p, install_neuronx_cc_hook, partition_id_tensor


class CompiledKernel:
    def __init__(self, nc, n_cores):
        install_neuronx_cc_hook()
        self.nc = nc
        self.n_cores = n_cores
        in_names, out_names, out_avals, zero_outs = [], [], [], []
        partition_name = nc.partition_id_tensor.name if nc.partition_id_tensor else None
        for alloc in nc.m.functions[0].allocations:
            if not isinstance(alloc, mybir.MemoryLocationSet):
                continue
            name = alloc.memorylocations[0].name
            if alloc.kind == "ExternalInput":
                if name != partition_name:
                    in_names.append(name)
            elif alloc.kind == "ExternalOutput":
                out_names.append(name)
                shape = tuple(alloc.tensor_shape)
                dtype = mybir.dt.np(alloc.dtype)
                out_avals.append(jax.core.ShapedArray(shape, dtype))
                zero_outs.append(np.zeros(shape, dtype))
        assert nc.dbg_addr is None
        self.in_names = list(in_names)
        self.out_names = out_names
        n_params = len(in_names)
        n_outs = len(out_avals)
        all_in_names = in_names + out_names + ([partition_name] if partition_name else [])

        def _body(*args):
            operands = list(args)
            if partition_name is not None:
                operands.append(partition_id_tensor())
            outs = _bass_exec_p.bind(
                *operands,
                out_avals=tuple(out_avals),
                in_names=tuple(all_in_names),
                out_names=tuple(out_names),
                lowering_input_output_aliases=(),
                sim_require_finite=True,
                sim_require_nnan=True,
                nc=nc,
            )
            return tuple(outs)

        devices = jax.devices()[:n_cores]
        mesh = Mesh(np.asarray(devices), ("core",))
        in_specs = (PartitionSpec("core"),) * (n_params + n_outs)
        out_specs = (PartitionSpec("core"),) * n_outs
        # NOTE: no donation so we can reuse the same zero-out buffers across calls.
        self.fn = jax.jit(
            shard_map(_body, mesh=mesh, in_specs=in_specs, out_specs=out_specs,
                      check_rep=False),
            keep_unused=True,
        )
        self.out_avals = out_avals
        self.zero_outs = zero_outs
        self.n_params = n_params

    def prep_inputs(self, in_maps):
        """in_maps: list of dict name->np array (one per core). Returns device args."""
        n = self.n_cores
        per_core = [[np.asarray(m[name]) for name in self.in_names] for m in in_maps]
        concat_in = [np.concatenate([per_core[c][i] for c in range(n)], axis=0)
                     for i in range(self.n_params)]
        concat_zeros = [np.zeros((n * z.shape[0], *z.shape[1:]), z.dtype)
                        for z in self.zero_outs]
        args = [jax.device_put(a) for a in concat_in + concat_zeros]
        jax.block_until_ready(args)
        return args

    def run(self, args):
        out = self.fn(*args)
        jax.block_until_ready(out)
        return out

    def results(self, out_arrs):
        n = self.n_cores
        res = []
        for c in range(n):
            res.append({name: np.asarray(out_arrs[i]).reshape(n, *self.out_avals[i].shape)[c]
                        for i, name in enumerate(self.out_names)})
        return res

    def time_it(self, args, iters=10, warmup=3):
        for _ in range(warmup):
            self.run(args)
        ts = []
        for _ in range(iters):
            t0 = time.perf_counter()
            self.run(args)
            ts.append(time.perf_counter() - t0)
        return min(ts), float(np.median(ts))

# ---------------- host-side prep (hardcoded shapes/sharding) ----------------
N_IMG, H_IMG = 4, 96
f16h = np.float16

def _prep_core(x, n, half):
    r0 = half * OH
    xp = np.zeros((C, RH, WP), np.float32)
    lo = max(0, r0 - PAD); hi = min(H_IMG, r0 + OH + PAD)
    xp[:, (lo - (r0 - PAD)):(hi - (r0 - PAD)), PAD:PAD + W] = x[n, :, lo:hi, :]
    xcp = xp.reshape(C, V).astype(f16h)
    xcl = np.ascontiguousarray(xp.transpose(1, 2, 0)).astype(f16h).reshape(V, C)
    return xcp, xcl

def _prep_weights(offset_w, offset_b, mod_w, mod_b, weight, bias, gamma, beta):
    perm = list(range(0, 18, 2)) + list(range(1, 18, 2))
    ow = offset_w[perm]; ob = offset_b[perm]
    aw = np.concatenate([ow, mod_w], 0)
    waux = np.zeros((C, KK * 27), np.float32)
    for t in range(KK):
        ty, tx = t // 3, t % 3
        waux[:, t * 27:(t + 1) * 27] = aw[:, :, ty, tx].T
    wbase = np.zeros((3, 27), np.float32)
    for j in range(9):
        ky, kx = j // 3 - 1, j % 3 - 1
        wbase[0, j] = 1.0; wbase[2, j] = ky + ob[j]
        wbase[1, 9 + j] = 1.0; wbase[2, 9 + j] = kx + ob[9 + j]
        wbase[2, 18 + j] = mod_b[j]
    w2 = np.zeros((C, KK * C), np.float32)
    for t in range(KK):
        w2[:, t * C:(t + 1) * C] = 2.0 * weight[:, :, t // 3, t % 3].T
    hh, ww = np.meshgrid(np.arange(OH, dtype=np.float32),
                         np.arange(WP, dtype=np.float32) - PAD, indexing="ij")
    grid = np.stack([hh.reshape(-1), ww.reshape(-1),
                     np.ones(OH * WP, np.float32)], 0)
    scal = np.stack([gamma, beta], 1).astype(np.float32)
    return (waux.astype(f16h), wbase.astype(f16h), w2.astype(f16h),
            grid.astype(f16h), scal)

_CACHE = {}

def kernel(x, offset_w, offset_b, mod_w, mod_b, weight, bias, gamma, beta):
    """Full-input DCNv2 -> BN -> ReLU on 8 NeuronCores (batch x half-image
    data parallel; BN stats all-reduced on device)."""
    waux, wbase, w2, grid, scal = _prep_weights(
        np.asarray(offset_w, np.float32), np.asarray(offset_b, np.float32),
        np.asarray(mod_w, np.float32), np.asarray(mod_b, np.float32),
        np.asarray(weight, np.float32), np.asarray(bias, np.float32),
        np.asarray(gamma, np.float32), np.asarray(beta, np.float32))
    x = np.asarray(x, np.float32)
    in_maps = []
    for n in range(N_IMG):
        for half in (0, 1):
            xcp, xcl = _prep_core(x, n, half)
            in_maps.append(dict(xcl=xcl, xcp=xcp, waux=waux, wbase=wbase,
                                w2=w2, grid=grid, scal=scal))
    if "k" not in _CACHE:
        nc = build(n_cores=8, do_collective=True)
        _CACHE["k"] = CompiledKernel(nc, 8)
    k = _CACHE["k"]
    args = k.prep_inputs(in_maps)
    results = k.results(k.run(args))
    out = np.zeros((N_IMG, C, H_IMG, H_IMG), np.float32)
    i = 0
    for n in range(N_IMG):
        for half in (0, 1):
            out[n, :, half * OH:(half + 1) * OH, :] =                 results[i]["out"].reshape(C, OH, W)
            i += 1
    return out

